# revision 1
# baseline (speedup 1.0000x reference)
"""DeepseekV3 MLA attention (B=2, S=2048, D=2048, H=16) on 8 trn2 NeuronCores.

Sharding: data-parallel over batch x tensor-parallel over heads.
Core c handles batch b=c//4 and heads [4*(c%4) .. 4*(c%4)+4).

Per-core device pipeline (fp16 matmul operands, fp32 PSUM accumulation):
  stage A (token-sharded: each core computes its own 512-token stile for its
  batch, then the 4 cores of a batch group AllGather the normalized
  low-rank activations):
    hiddenT stile (host-transposed f32) -> cast f16
    q_aT = wqa^T-contract, ckvT = wkva^T-contract     (T layout [feat, tok])
    RMSNorm in T layout (sumsq via ones-matmul, rsqrt, K=1 broadcast matmul)
  stage B (on gathered activations, all 2048 tokens):
    qTn/qTr/kTn in T layout, V in natural layout
  RoPE in T layout with host-precomputed cos/sin tables.
  Attention computed TRANSPOSED per k-tile: scoresT[k,q]; exp on ACT with no
  max subtraction (logit range ~[-4,4] for this distribution); causal
  masking via precomputed 0/1 tiles; denominators via ones-matmul;
  PV with PT as moving operand -> attnT[dv,q]; normalize via K=1 broadcast
  of reciprocal row sums.
  o-proj partials over local heads -> chunked ReduceScatter(add) within the
  batch group -> each core outputs its own 512-token slice of the output.

Host side only shards/transposes/concats (weight folding of the RMSNorm
gains and the softmax scale is compile-time weight prep).
"""

import numpy as np

import concourse.bass as bass
import concourse.mybir as mybir
import concourse.tile as tile
from concourse.bass_utils import run_bass_kernel_spmd

F32 = mybir.dt.float32
F16 = mybir.dt.float16
AF = mybir.ActivationFunctionType

B, S, D = 2, 2048, 2048
H = 16
NOPE, ROPE, VDIM = 128, 64, 128
QHD = NOPE + ROPE
QR, KVR = 1536, 512
THETA = 10000.0
EPS = 1e-6
SCALE = QHD ** -0.5

HPG = 4          # heads per group (per core)
NST = 4          # 512-token stiles
ST = 512
NDC = D // 128   # 16 d-chunks
NRC = QR // 128  # 12 rank chunks (q)
NKC = KVR // 128 # 4 rank chunks (kv)
NTT = S // 128   # 16 token tiles
GROUPS = [[0, 1, 2, 3], [4, 5, 6, 7]]


def _split_multi_waits(nc):
    """walrus in this container accepts only ONE sem wait per instruction;
    split extras onto same-engine NOPs placed immediately before."""
    ctr = 0
    for bb in nc.main_func.blocks:
        new = []
        for ins in bb.instructions:
            si = ins.sync_info
            if si is not None and len(si.on_wait) > 1:
                waits = list(si.on_wait)
                for w in waits[:-1]:
                    nop = mybir.InstNoOp(name=f"I-ws{ctr}", ins=[], outs=[])
                    ctr += 1
                    nop.engine = ins.engine
                    nop.sync_info = mybir.SyncInfo(on_wait=[w], on_update=[])
                    new.append(nop)
                si.on_wait = [waits[-1]]
                ins.sync_info = si
            new.append(ins)
        bb.instructions = new


def _build_program(mask_mode):
    """mask_mode: 'causal' | 'none' | 'generic'"""
    nc = bass.Bass()

    hT_d = nc.dram_tensor("hiddenT", [D, ST], F32, kind="ExternalInput")
    wqa_d = nc.dram_tensor("wqa", [D, QR], F16, kind="ExternalInput")
    wkva_d = nc.dram_tensor("wkva", [D, KVR + ROPE], F16, kind="ExternalInput")
    wqbn_d = nc.dram_tensor("wqbn", [QR, HPG * NOPE], F16, kind="ExternalInput")
    wqbr_d = nc.dram_tensor("wqbr", [QR, HPG * ROPE], F16, kind="ExternalInput")
    wkvbk_d = nc.dram_tensor("wkvbk", [KVR, HPG * NOPE], F16, kind="ExternalInput")
    wkvbv_d = nc.dram_tensor("wkvbv", [KVR, HPG * VDIM], F16, kind="ExternalInput")
    wo_d = nc.dram_tensor("wo", [H * VDIM, D], F16, kind="ExternalInput")
    cos2_d = nc.dram_tensor("cos2", [2 * ROPE, S], F16, kind="ExternalInput")
    sin2_d = nc.dram_tensor("sin2", [2 * ROPE, S], F16, kind="ExternalInput")
    if mask_mode == "causal":
        pmask_d = nc.dram_tensor("pmaskT", [4, 128, ST], F16, kind="ExternalInput")
    if mask_mode == "generic":
        maskT_d = nc.dram_tensor("maskT", [S, S], F32, kind="ExternalInput")
    o_d = nc.dram_tensor("o_part", [ST, D], F32, kind="ExternalOutput")

    with tile.TileContext(nc) as tc:
        with (
            tc.tile_pool(name="const", bufs=1) as pco,
            tc.tile_pool(name="persist", bufs=1) as pp,
            tc.tile_pool(name="dram", bufs=1, space="DRAM") as pdr,
        ):
            ones_col = pco.tile([128, 1], F16)
            nc.vector.memset(ones_col[:], 1.0)
            ones_row = pco.tile([1, 128], F16)
            nc.vector.memset(ones_row[:], 1.0)
            epst = pco.tile([1, 1], F32)
            nc.vector.memset(epst[:], EPS)

            # persistent activation tensors
            qTn = [pp.tile([128, S], F16, name=f"qTn{i}", tag=f"qTn{i}") for i in range(HPG)]
            qTr_raw = [pp.tile([128, S], F16, name=f"qTrr{i}", tag=f"qTrr{i}") for i in range(2)]
            kTn = [pp.tile([128, S], F16, name=f"kTn{i}", tag=f"kTn{i}") for i in range(HPG)]
            Vn = [pp.tile([128, HPG * VDIM], F16, name=f"V{i}", tag=f"V{i}") for i in range(NTT)]
            kpe_raw = pp.tile([ROPE, S], F16)

            # DRAM bounce buffers for the activation AllGather (q rows, then
            # kv rows, then k_pe rows packed into one payload)
            AGR = QR + KVR + ROPE
            aga_src = pdr.tile([AGR, ST], F16, name="aga_src", tag="aga_src")
            aga_dst = pdr.tile([NST, AGR, ST], F16, name="aga_dst", tag="aga_dst")

            # ---------------- stage A: own stile only ----------------
            with (
                tc.tile_pool(name="wA", bufs=1) as pw,
                tc.tile_pool(name="loopA", bufs=2) as pl,
                tc.tile_pool(name="loopA1", bufs=1) as pl1,
                tc.tile_pool(name="rawA", bufs=1) as pr,
                tc.tile_pool(name="psA", bufs=3, space="PSUM") as psm,
                tc.tile_pool(name="psRow", bufs=2, space="PSUM") as psr,
            ):
                # hidden stile first (critical path; SWDGE casts f32->f16
                # during the DMA), then A weights
                ht = []
                for dc in range(NDC):
                    h16 = pr.tile([128, ST], F16, name=f"ht{dc}", tag=f"ht{dc}")
                    nc.gpsimd.dma_start(h16[:], hT_d[dc * 128:(dc + 1) * 128, :])
                    ht.append(h16)
                wqa = [pw.tile([128, QR], F16, name=f"wqa{dc}", tag=f"wqa{dc}") for dc in range(NDC)]
                for dc in range(NDC):
                    nc.sync.dma_start(wqa[dc][:], wqa_d[dc * 128:(dc + 1) * 128, :])
                wkva = [pw.tile([128, KVR + ROPE], F16, name=f"wkva{dc}", tag=f"wkva{dc}") for dc in range(NDC)]
                for dc in range(NDC):
                    nc.sync.dma_start(wkva[dc][:], wkva_d[dc * 128:(dc + 1) * 128, :])

                # ---- A-proj q + rms ----
                qraw = []
                pss = psr.tile([1, ST], F32, name="pss", tag="pss")
                for rc in range(NRC):
                    ps = psm.tile([128, ST], F32, name="psA", tag="psA")
                    for dc in range(NDC):
                        nc.tensor.matmul(
                            ps[:], wqa[dc][:, rc * 128:(rc + 1) * 128], ht[dc][:],
                            start=(dc == 0), stop=(dc == NDC - 1))
                    raw = pr.tile([128, ST], F16, name=f"qraw{rc}", tag=f"qraw{rc}")
                    nc.any.tensor_copy(raw[:], ps[:])
                    qraw.append(raw)
                    sq = pl.tile([128, ST], F16, name="sq", tag="sq")
                    nc.vector.tensor_mul(sq[:], raw[:], raw[:])
                    nc.tensor.matmul(pss[:], ones_col[:], sq[:],
                                     start=(rc == 0), stop=(rc == NRC - 1))
                sqv = pl1.tile([1, ST], F32, name="sqv", tag="sqv")
                nc.scalar.activation(sqv[:], pss[:], AF.Sqrt, scale=1.0 / QR, bias=epst[:])
                inv = pl1.tile([1, ST], F32, name="inv", tag="inv")
                nc.vector.reciprocal(inv[:], sqv[:])
                inv16 = pl1.tile([1, ST], F16, name="inv16", tag="inv16")
                nc.any.tensor_copy(inv16[:], inv[:])
                psb = psm.tile([128, ST], F32, name="psA", tag="psA")
                nc.tensor.matmul(psb[:], ones_row[:], inv16[:], start=True, stop=True)
                bch = pl1.tile([128, ST], F16, name="bch", tag="bch")
                nc.any.tensor_copy(bch[:], psb[:])
                for rc in range(NRC):
                    nc.vector.tensor_mul(qraw[rc][:], qraw[rc][:], bch[:])
                    nc.sync.dma_start(aga_src[rc * 128:(rc + 1) * 128, :], qraw[rc][:])

                # ---- A-proj ckv + rms; rope part raw ----
                kraw = []
                pss2 = psr.tile([1, ST], F32, name="pss", tag="pss")
                for rc in range(NKC):
                    ps = psm.tile([128, ST], F32, name="psA", tag="psA")
                    for dc in range(NDC):
                        nc.tensor.matmul(
                            ps[:], wkva[dc][:, rc * 128:(rc + 1) * 128], ht[dc][:],
                            start=(dc == 0), stop=(dc == NDC - 1))
                    raw = pr.tile([128, ST], F16, name=f"kraw{rc}", tag=f"kraw{rc}")
                    nc.any.tensor_copy(raw[:], ps[:])
                    kraw.append(raw)
                    sq = pl.tile([128, ST], F16, name="sq", tag="sq")
                    nc.vector.tensor_mul(sq[:], raw[:], raw[:])
                    nc.tensor.matmul(pss2[:], ones_col[:], sq[:],
                                     start=(rc == 0), stop=(rc == NKC - 1))
                psp = psm.tile([ROPE, ST], F32, name="psRope", tag="psRope", bufs=1)
                for dc in range(NDC):
                    nc.tensor.matmul(psp[:], wkva[dc][:, KVR:KVR + ROPE], ht[dc][:],
                                     start=(dc == 0), stop=(dc == NDC - 1))
                kpe_s = pl1.tile([ROPE, ST], F16, name="kpe_s", tag="kpe_s")
                nc.any.tensor_copy(kpe_s[:], psp[:])
                nc.sync.dma_start(aga_src[QR + KVR:AGR, :], kpe_s[:])

                sqv2 = pl1.tile([1, ST], F32, name="sqv", tag="sqv")
                nc.scalar.activation(sqv2[:], pss2[:], AF.Sqrt, scale=1.0 / KVR, bias=epst[:])
                inv2 = pl1.tile([1, ST], F32, name="inv", tag="inv")
                nc.vector.reciprocal(inv2[:], sqv2[:])
                inv162 = pl1.tile([1, ST], F16, name="inv16", tag="inv16")
                nc.any.tensor_copy(inv162[:], inv2[:])
                psb2 = psm.tile([128, ST], F32, name="psA", tag="psA")
                nc.tensor.matmul(psb2[:], ones_row[:], inv162[:], start=True, stop=True)
                bch2 = pl1.tile([128, ST], F16, name="bch", tag="bch")
                nc.any.tensor_copy(bch2[:], psb2[:])
                for rc in range(NKC):
                    nc.vector.tensor_mul(kraw[rc][:], kraw[rc][:], bch2[:])
                    nc.sync.dma_start(aga_src[QR + rc * 128:QR + (rc + 1) * 128, :], kraw[rc][:])

            # ---- AllGather the normalized low-rank activations ----
            nc.gpsimd.collective_compute(
                "AllGather", mybir.AluOpType.bypass, replica_groups=GROUPS,
                ins=[aga_src.opt()], outs=[aga_dst.opt()])

            # ---------------- stage B on gathered activations ----------------
            with (
                tc.tile_pool(name="wB", bufs=1) as pwb,
                tc.tile_pool(name="gath", bufs=1) as pg,
                tc.tile_pool(name="psB", bufs=1, space="PSUM") as psmb,
            ):
                wqbn = [pwb.tile([128, HPG * NOPE], F16, name=f"wqbn{rc}", tag=f"wqbn{rc}") for rc in range(NRC)]
                wqbr = [pwb.tile([128, HPG * ROPE], F16, name=f"wqbr{rc}", tag=f"wqbr{rc}") for rc in range(NRC)]
                for rc in range(NRC):
                    nc.sync.dma_start(wqbn[rc][:], wqbn_d[rc * 128:(rc + 1) * 128, :])
                    nc.sync.dma_start(wqbr[rc][:], wqbr_d[rc * 128:(rc + 1) * 128, :])
                wkvbk = [pwb.tile([128, HPG * NOPE], F16, name=f"wkvbk{rc}", tag=f"wkvbk{rc}") for rc in range(NKC)]
                wkvbv = [pwb.tile([128, HPG * VDIM], F16, name=f"wkvbv{rc}", tag=f"wkvbv{rc}") for rc in range(NKC)]
                for rc in range(NKC):
                    nc.sync.dma_start(wkvbk[rc][:], wkvbk_d[rc * 128:(rc + 1) * 128, :])
                    nc.sync.dma_start(wkvbv[rc][:], wkvbv_d[rc * 128:(rc + 1) * 128, :])

                # load gathered activations (all stiles resident)
                qg = [[pg.tile([128, ST], F16, name=f"qg{s}_{rc}", tag=f"qg{s}_{rc}")
                       for rc in range(NRC)] for s in range(NST)]
                kg = [[pg.tile([128, ST], F16, name=f"kg{s}_{rc}", tag=f"kg{s}_{rc}")
                       for rc in range(NKC)] for s in range(NST)]
                for s in range(NST):
                    for rc in range(NRC):
                        nc.sync.dma_start(qg[s][rc][:], aga_dst[s, rc * 128:(rc + 1) * 128, :])
                    for rc in range(NKC):
                        nc.sync.dma_start(kg[s][rc][:], aga_dst[s, QR + rc * 128:QR + (rc + 1) * 128, :])
                    nc.sync.dma_start(kpe_raw[:, s * ST:(s + 1) * ST],
                                      aga_dst[s, QR + KVR:AGR, :])

                # per output chunk keep 4 per-stile psums alive so consecutive
                # matmuls share the same stationary operand
                for mc in range(HPG):
                    pss4 = [psmb.tile([128, ST], F32, name=f"psB{s}", tag=f"psB{s}", bufs=1)
                            for s in range(NST)]
                    for rc in range(NRC):
                        for s in range(NST):
                            nc.tensor.matmul(
                                pss4[s][:], wqbn[rc][:, mc * 128:(mc + 1) * 128], qg[s][rc][:],
                                start=(rc == 0), stop=(rc == NRC - 1))
                    for s in range(NST):
                        nc.any.tensor_copy(qTn[mc][:, s * ST:(s + 1) * ST], pss4[s][:])
                for mc in range(2):
                    pss4 = [psmb.tile([128, ST], F32, name=f"psB{s}", tag=f"psB{s}", bufs=1)
                            for s in range(NST)]
                    for rc in range(NRC):
                        for s in range(NST):
                            nc.tensor.matmul(
                                pss4[s][:], wqbr[rc][:, mc * 128:(mc + 1) * 128], qg[s][rc][:],
                                start=(rc == 0), stop=(rc == NRC - 1))
                    for s in range(NST):
                        nc.any.tensor_copy(qTr_raw[mc][:, s * ST:(s + 1) * ST], pss4[s][:])
                for mc in range(HPG):
                    pss4 = [psmb.tile([128, ST], F32, name=f"psB{s}", tag=f"psB{s}", bufs=1)
                            for s in range(NST)]
                    for rc in range(NKC):
                        for s in range(NST):
                            nc.tensor.matmul(
                                pss4[s][:], wkvbk[rc][:, mc * 128:(mc + 1) * 128], kg[s][rc][:],
                                start=(rc == 0), stop=(rc == NKC - 1))
                    for s in range(NST):
                        nc.any.tensor_copy(kTn[mc][:, s * ST:(s + 1) * ST], pss4[s][:])
                for s in range(NST):
                    for tt in range(4):
                        ps = psmb.tile([128, HPG * VDIM], F32, name="psB0", tag="psB0", bufs=1)
                        for rc in range(NKC):
                            nc.tensor.matmul(
                                ps[:], kg[s][rc][:, tt * 128:(tt + 1) * 128], wkvbv[rc][:],
                                start=(rc == 0), stop=(rc == NKC - 1))
                        nc.any.tensor_copy(Vn[s * 4 + tt][:], ps[:])

            # ---------------- RoPE ----------------
            post_pool = tc.tile_pool(name="post", bufs=1)
            pp2 = post_pool.__enter__()
            qTr = [pp2.tile([128, S], F16, name=f"qTr{i}", tag=f"qTr{i}") for i in range(2)]
            kpe = pp2.tile([ROPE, S], F16)
            with tc.tile_pool(name="rope", bufs=1) as pro:
                cos2 = pco.tile([2 * ROPE, S], F16)
                nc.sync.dma_start(cos2[:], cos2_d[:])
                sin2 = pco.tile([2 * ROPE, S], F16)
                nc.sync.dma_start(sin2[:], sin2_d[:])
                HR = ROPE // 2  # 32
                # k side
                rot = pro.tile([ROPE, S], F16, name="rotk", tag="rotk")
                nc.vector.tensor_scalar_mul(rot[0:HR, :], kpe_raw[HR:ROPE, :], -1.0)
                nc.vector.tensor_copy(rot[HR:ROPE, :], kpe_raw[0:HR, :])
                t1 = pro.tile([ROPE, S], F16, name="t1k", tag="t1k")
                nc.vector.tensor_mul(t1[:], kpe_raw[:], cos2[0:ROPE, :])
                t2 = pro.tile([ROPE, S], F16, name="t2k", tag="t2k")
                nc.vector.tensor_mul(t2[:], rot[:], sin2[0:ROPE, :])
                nc.vector.tensor_add(kpe[:], t1[:], t2[:])
                # q side (2 tiles, each = 2 heads x 64 rows)
                for i in range(2):
                    rq = pro.tile([128, S], F16, name="rotq", tag="rotq")
                    for hh in range(2):
                        o = hh * ROPE
                        nc.vector.tensor_scalar_mul(
                            rq[o:o + HR, :], qTr_raw[i][o + HR:o + ROPE, :], -1.0)
                        nc.vector.tensor_copy(
                            rq[o + HR:o + ROPE, :], qTr_raw[i][o:o + HR, :])
                    u1 = pro.tile([128, S], F16, name="u1", tag="u1")
                    nc.vector.tensor_mul(u1[:], qTr_raw[i][:], cos2[:])
                    u2 = pro.tile([128, S], F16, name="u2", tag="u2")
                    nc.vector.tensor_mul(u2[:], rq[:], sin2[:])
                    nc.vector.tensor_add(qTr[i][:], u1[:], u2[:])

            # kpe duplicated into both partition halves so the rope matmul's
            # lhsT base_partition can match either q-rope slice (0 or 64)
            kpe_both = pp2.tile([128, S], F16)
            nc.vector.tensor_copy(kpe_both[0:ROPE, :], kpe[:])
            nc.vector.tensor_copy(kpe_both[ROPE:2 * ROPE, :], kpe[:])

            # ---------------- attention (transposed) ----------------
            attnT = [pp2.tile([128, S], F16, name=f"attnT{i}", tag=f"attnT{i}") for i in range(HPG)]
            with (
                tc.tile_pool(name="attn", bufs=1) as pat,
                tc.tile_pool(name="ptp", bufs=6) as ptp,
                tc.tile_pool(name="psS", bufs=3, space="PSUM") as psS,
                tc.tile_pool(name="psR", bufs=2, space="PSUM") as psR,
                tc.tile_pool(name="psA2", bufs=2, space="PSUM") as psA2,
            ):
                if mask_mode == "causal":
                    pmask = [pat.tile([128, ST], F16, name=f"pm{r}", tag=f"pm{r}") for r in range(4)]
                    for r in range(4):
                        nc.sync.dma_start(pmask[r][:], pmask_d[r])
                for h in range(HPG):
                    qtr_t = qTr[h // 2]
                    ro = (h % 2) * ROPE
                    for qb in range(NST):
                        qsl = slice(qb * ST, (qb + 1) * ST)
                        nkt = 4 * (qb + 1) if mask_mode == "causal" else NTT
                        ps_rs = psR.tile([1, ST], F32, name="psrs", tag="psrs")
                        ps_at = psA2.tile([128, ST], F32, name="psat", tag="psat")
                        for kt in range(nkt):
                            ps = psS.tile([128, ST], F32, name="pss", tag="pss")
                            ksl = slice(kt * 128, (kt + 1) * 128)
                            nc.tensor.matmul(ps[:], kTn[h][:, ksl], qTn[h][:, qsl],
                                             start=True, stop=False)
                            nc.tensor.matmul(ps[:], kpe_both[ro:ro + ROPE, ksl],
                                             qtr_t[ro:ro + ROPE, qsl],
                                             start=False, stop=True)
                            if mask_mode == "generic":
                                mt = ptp.tile([128, ST], F32, name="mt", tag="mt")
                                nc.sync.dma_start(mt[:], maskT_d[ksl, qsl])
                                nc.vector.tensor_add(ps[:], ps[:], mt[:])
                            pt = ptp.tile([128, ST], F16, name="pt", tag="pt")
                            nc.scalar.activation(pt[:], ps[:], AF.Exp)
                            if mask_mode == "causal" and kt >= 4 * qb:
                                nc.vector.tensor_mul(pt[:], pt[:], pmask[kt % 4][:])
                            nc.tensor.matmul(ps_rs[:], ones_col[:], pt[:],
                                             start=(kt == 0), stop=(kt == nkt - 1))
                            nc.tensor.matmul(ps_at[:], Vn[kt][:, h * VDIM:(h + 1) * VDIM],
                                             pt[:], start=(kt == 0), stop=(kt == nkt - 1))
                        invr = pat.tile([1, ST], F32, name="invr", tag="invr")
                        nc.vector.reciprocal(invr[:], ps_rs[:])
                        invr16 = pat.tile([1, ST], F16, name="invr16", tag="invr16")
                        nc.any.tensor_copy(invr16[:], invr[:])
                        psb = psS.tile([128, ST], F32, name="pss", tag="pss")
                        nc.tensor.matmul(psb[:], ones_row[:], invr16[:], start=True, stop=True)
                        bc16 = pat.tile([128, ST], F16, name="bc16", tag="bc16")
                        nc.any.tensor_copy(bc16[:], psb[:])
                        nc.vector.tensor_mul(attnT[h][:, qsl], ps_at[:], bc16[:])

            # ------- o-proj: AllGather attnT, slice own tokens, full contract -------
            agat_src = pdr.tile([HPG * VDIM, S], F16, name="agat_src", tag="agat_src")
            agat_dst = pdr.tile([NST, HPG * VDIM, S], F16, name="agat_dst", tag="agat_dst")
            for hc in range(HPG):
                nc.sync.dma_start(agat_src[hc * 128:(hc + 1) * 128, :], attnT[hc][:])
            nc.gpsimd.collective_compute(
                "AllGather", mybir.AluOpType.bypass, replica_groups=GROUPS,
                ins=[agat_src.opt()], outs=[agat_dst.opt()])
            with (
                tc.tile_pool(name="oproj", bufs=1) as po,
                tc.tile_pool(name="oloop", bufs=3) as pol,
                tc.tile_pool(name="psO", bufs=2, space="PSUM") as psO,
            ):
                pid = nc.partition_id()
                toff = nc.snap((pid % NST) * ST, donate=True)
                wo = [po.tile([128, D], F16, name=f"wo{hc}", tag=f"wo{hc}") for hc in range(H)]
                for hc in range(H):
                    nc.sync.dma_start(wo[hc][:], wo_d[hc * 128:(hc + 1) * 128, :])
                atg = [po.tile([128, ST], F16, name=f"atg{hc}", tag=f"atg{hc}") for hc in range(H)]
                for hc in range(H):
                    nc.gpsimd.dma_start(
                        atg[hc][:],
                        agat_dst[hc // 4, (hc % 4) * 128:(hc % 4 + 1) * 128,
                                 bass.ds(toff, ST)])
                for ncol in range(4):
                    csl = slice(ncol * ST, (ncol + 1) * ST)
                    for tl in range(4):
                        ps = psO.tile([128, ST], F32, name="pso", tag="pso")
                        for hc in range(H):
                            nc.tensor.matmul(ps[:], atg[hc][:, tl * 128:(tl + 1) * 128],
                                             wo[hc][:, csl],
                                             start=(hc == 0), stop=(hc == H - 1))
                        ot = pol.tile([128, ST], F32, name="ot", tag="ot")
                        nc.any.tensor_copy(ot[:], ps[:])
                        nc.sync.dma_start(o_d[tl * 128:(tl + 1) * 128, csl], ot[:])
            post_pool.__exit__(None, None, None)

    _split_multi_waits(nc)
    return nc


_CACHE = {}


def _get_program(mask_mode):
    if mask_mode not in _CACHE:
        _CACHE[mask_mode] = _build_program(mask_mode)
    return _CACHE[mask_mode]


def _host_prep(hidden_states, attention_mask, position_ids, w_qa, qa_ln_w, w_qb,
               w_kva, kva_ln_w, w_kvb, w_o):
    f16 = np.float16
    mask2d = np.asarray(attention_mask, np.float32).reshape(S, S)
    causal_ref = np.triu(np.full((S, S), -1e9, np.float32), k=1)
    if np.array_equal(mask2d, causal_ref):
        mask_mode = "causal"
    elif not mask2d.any():
        mask_mode = "none"
    else:
        mask_mode = "generic"

    # weight prep: fold RMSNorm gains into B-projections, SCALE into q side
    w_qb_eff = (np.asarray(w_qb, np.float32) * np.asarray(qa_ln_w, np.float32)[:, None]) * SCALE
    w_kvb_eff = np.asarray(w_kvb, np.float32) * np.asarray(kva_ln_w, np.float32)[:, None]
    wqb3 = w_qb_eff.reshape(QR, H, QHD)
    wkvb3 = w_kvb_eff.reshape(KVR, H, NOPE + VDIM)
    w_o3 = np.asarray(w_o, np.float32).reshape(H, VDIM, D)

    pos = np.asarray(position_ids).astype(np.int64)
    inv_freq = 1.0 / (THETA ** (np.arange(0, ROPE, 2, dtype=np.float32) / ROPE))
    t = np.arange(S, dtype=np.float32)
    freqs = np.outer(t, inv_freq)
    emb = np.concatenate([freqs, freqs], axis=-1)   # [S, ROPE]
    cosT = np.cos(emb)[pos].T.astype(f16)           # [ROPE, S]
    sinT = np.sin(emb)[pos].T.astype(f16)
    cos2 = np.ascontiguousarray(np.concatenate([cosT, cosT], axis=0))  # [128, S]
    sin2 = np.ascontiguousarray(np.concatenate([sinT, sinT], axis=0))

    # causal keep-mask patterns for the transposed diagonal tiles:
    # keep iff 128*r + ki <= qj  (r = kt % 4)
    ki = np.arange(128)[:, None]
    qj = np.arange(ST)[None, :]
    pmaskT = np.stack([(128 * r + ki <= qj) for r in range(4)]).astype(f16)

    wqa16 = np.asarray(w_qa, np.float32).astype(f16)
    wkva16 = np.asarray(w_kva, np.float32).astype(f16)

    hiddenT = [np.ascontiguousarray(np.asarray(hidden_states[b], np.float32).T)
               for b in range(B)]
    wo_full = np.asarray(w_o, np.float32).astype(f16)

    in_maps = []
    for c in range(8):
        b, g = divmod(c, 4)
        hs = range(g * HPG, (g + 1) * HPG)
        m = {
            "hiddenT": np.ascontiguousarray(hiddenT[b][:, g * ST:(g + 1) * ST]),
            "wqa": wqa16,
            "wkva": wkva16,
            "wqbn": np.ascontiguousarray(
                np.concatenate([wqb3[:, h, :NOPE] for h in hs], axis=1)).astype(f16),
            "wqbr": np.ascontiguousarray(
                np.concatenate([wqb3[:, h, NOPE:] for h in hs], axis=1)).astype(f16),
            "wkvbk": np.ascontiguousarray(
                np.concatenate([wkvb3[:, h, :NOPE] for h in hs], axis=1)).astype(f16),
            "wkvbv": np.ascontiguousarray(
                np.concatenate([wkvb3[:, h, NOPE:] for h in hs], axis=1)).astype(f16),
            "wo": wo_full,
            "cos2": cos2,
            "sin2": sin2,
        }
        if mask_mode == "causal":
            m["pmaskT"] = pmaskT
        if mask_mode == "generic":
            m["maskT"] = np.ascontiguousarray(mask2d.T)
        in_maps.append(m)
    return mask_mode, in_maps


def kernel(hidden_states, attention_mask, position_ids, w_qa, qa_ln_w, w_qb,
           w_kva, kva_ln_w, w_kvb, w_o, _want_trace=False, _trace_kwargs=None):
    mask_mode, in_maps = _host_prep(
        hidden_states, attention_mask, position_ids, w_qa, qa_ln_w, w_qb,
        w_kva, kva_ln_w, w_kvb, w_o)
    nc = _get_program(mask_mode)
    kwargs = {}
    if _want_trace:
        kwargs.update(trace=True, **(_trace_kwargs or {}))
    res = run_bass_kernel_spmd(nc, in_maps, list(range(8)), **kwargs)
    out = np.empty((B, S, D), np.float32)
    for c in range(8):
        b, g = divmod(c, 4)
        out[b, g * ST:(g + 1) * ST, :] = res.results[c]["o_part"]
    if _want_trace:
        kernel._last_result = res
    return out



# revision 6
# speedup vs baseline: 1.4091x; 1.4091x over previous
"""DeepseekV3 MLA attention (B=2, S=2048, D=2048, H=16) on 8 trn2 NeuronCores.

Sharding: data-parallel over batch x tensor-parallel over heads.
Core c handles batch b=c//4 and heads [4*(c%4) .. 4*(c%4)+4).

v2 pipeline (vs baseline): collectives are split and overlapped with compute.
  stage A (token-sharded): kv A-proj first -> AG of RAW kv activations
  (+k_pe rows +inv_kv row) issued early, overlapping the q A-proj; then AG of
  RAW q activations (+inv_q row).  RMSNorm inv factors travel with the AG and
  are applied AFTER the B-projections (per-token column scaling commutes with
  the rank contraction), so stage A never serializes on the norm.
  stage B: kv-side work (kTn, V, k-rope) runs under the q AllGather; q_b
  follows when the q AG lands.
  attention: transposed per k-tile as baseline, but the softmax-normalize
  chain is ones-broadcast-matmul -> reciprocal_approx_fast on the [128,512]
  broadcast -> one fused multiply (no serial [1,512] reciprocal), with a
  dedicated PSUM bank for the broadcast so consecutive (head, q-block) units
  pipeline without stalling the PE.
  After each head finishes, its attnT is AllGathered immediately (4 chunked
  collectives hidden under the remaining heads' attention compute).
  o-proj: full contract over 16 gathered heads for this core's own 512-token
  slice, as baseline.

Host side only shards/transposes/concats (weight folding of the RMSNorm
gains and the softmax scale is compile-time weight prep).
"""

import numpy as np

import concourse.bass as bass
import concourse.mybir as mybir
import concourse.tile as tile
from concourse.bass_utils import run_bass_kernel_spmd

F32 = mybir.dt.float32
F16 = mybir.dt.float16
AF = mybir.ActivationFunctionType

B, S, D = 2, 2048, 2048
H = 16
NOPE, ROPE, VDIM = 128, 64, 128
QHD = NOPE + ROPE
QR, KVR = 1536, 512
THETA = 10000.0
EPS = 1e-6
SCALE = QHD ** -0.5

HPG = 4          # heads per group (per core)
NST = 4          # 512-token stiles
ST = 512
NDC = D // 128   # 16 d-chunks
NRC = QR // 128  # 12 rank chunks (q)
NKC = KVR // 128 # 4 rank chunks (kv)
NTT = S // 128   # 16 token tiles
GROUPS = [[0, 1, 2, 3], [4, 5, 6, 7]]
AGKV_R = KVR + ROPE + 1   # raw ckv rows + k_pe rows + inv_kv row
AGQ_R = QR + 1            # raw q rows + inv_q row


def _split_multi_waits(nc):
    """walrus in this container accepts only ONE sem wait per instruction;
    split extras onto same-engine NOPs placed immediately before."""
    ctr = 0
    for bb in nc.main_func.blocks:
        new = []
        for ins in bb.instructions:
            si = ins.sync_info
            if si is not None and len(si.on_wait) > 1:
                waits = list(si.on_wait)
                for w in waits[:-1]:
                    nop = mybir.InstNoOp(name=f"I-ws{ctr}", ins=[], outs=[])
                    ctr += 1
                    nop.engine = ins.engine
                    nop.sync_info = mybir.SyncInfo(on_wait=[w], on_update=[])
                    new.append(nop)
                si.on_wait = [waits[-1]]
                ins.sync_info = si
            new.append(ins)
        bb.instructions = new


def _build_program(mask_mode):
    """mask_mode: 'causal' | 'none' | 'generic'"""
    nc = bass.Bass()

    hT_d = nc.dram_tensor("hiddenT", [D, ST], F32, kind="ExternalInput")
    wqa_d = nc.dram_tensor("wqa", [D, QR], F16, kind="ExternalInput")
    wkva_d = nc.dram_tensor("wkva", [D, KVR + ROPE], F16, kind="ExternalInput")
    wqbn_d = nc.dram_tensor("wqbn", [QR, HPG * NOPE], F16, kind="ExternalInput")
    wqbr_d = nc.dram_tensor("wqbr", [QR, HPG * ROPE], F16, kind="ExternalInput")
    wkvbk_d = nc.dram_tensor("wkvbk", [KVR, HPG * NOPE], F16, kind="ExternalInput")
    wkvbv_d = nc.dram_tensor("wkvbv", [KVR, HPG * VDIM], F16, kind="ExternalInput")
    wo_d = nc.dram_tensor("wo", [H * VDIM, D], F16, kind="ExternalInput")
    cos2_d = nc.dram_tensor("cos2", [2 * ROPE, S], F16, kind="ExternalInput")
    sin2_d = nc.dram_tensor("sin2", [2 * ROPE, S], F16, kind="ExternalInput")
    if mask_mode == "causal":
        pmask_d = nc.dram_tensor("pmaskT", [4, 128, ST], F16, kind="ExternalInput")
    if mask_mode == "generic":
        maskT_d = nc.dram_tensor("maskT", [S, S], F32, kind="ExternalInput")
    o_d = nc.dram_tensor("o_part", [ST, D], F32, kind="ExternalOutput")

    with tile.TileContext(nc) as tc:
        with (
            tc.tile_pool(name="const", bufs=1) as pco,
            tc.tile_pool(name="persist", bufs=1) as pp,
            tc.tile_pool(name="dram", bufs=1, space="DRAM") as pdr,
        ):
            ones_col = pco.tile([128, 1], F16)
            nc.vector.memset(ones_col[:], 1.0)
            ones_row = pco.tile([1, 128], F16)
            nc.vector.memset(ones_row[:], 1.0)
            epst = pco.tile([1, 1], F32)
            nc.vector.memset(epst[:], EPS)

            # persistent activation tensors
            qTn = [pp.tile([128, S], F16, name=f"qTn{i}", tag=f"qTn{i}") for i in range(HPG)]
            qTr_raw = [pp.tile([128, S], F16, name=f"qTrr{i}", tag=f"qTrr{i}") for i in range(2)]
            kTn = [pp.tile([128, S], F16, name=f"kTn{i}", tag=f"kTn{i}") for i in range(HPG)]
            Vn = [pp.tile([128, HPG * VDIM], F16, name=f"V{i}", tag=f"V{i}") for i in range(NTT)]
            kpe_raw = pp.tile([ROPE, S], F16)

            # DRAM bounce buffers for the two activation AllGathers
            agkv_src = pdr.tile([AGKV_R, ST], F16, name="agkv_src", tag="agkv_src")
            agkv_dst = pdr.tile([NST, AGKV_R, ST], F16, name="agkv_dst", tag="agkv_dst")
            agq_src = pdr.tile([AGQ_R, ST], F16, name="agq_src", tag="agq_src")
            agq_dst = pdr.tile([NST, AGQ_R, ST], F16, name="agq_dst", tag="agq_dst")

            # ---------------- stage A: own stile only, RAW + inv rows ----------------
            with (
                tc.tile_pool(name="wA", bufs=1) as pw,
                tc.tile_pool(name="loopA", bufs=2) as pl,
                tc.tile_pool(name="loopA1", bufs=2) as pl1,
                tc.tile_pool(name="rawA", bufs=3) as pr,
                tc.tile_pool(name="psA", bufs=3, space="PSUM") as psm,
                tc.tile_pool(name="psRow", bufs=2, space="PSUM") as psr,
            ):
                # hidden stile first (critical path; SWDGE casts f32->f16
                # during the DMA), then kv-A weights (kv path runs first),
                # then q-A weights
                ht = []
                for dc in range(NDC):
                    h16 = pw.tile([128, ST], F16, name=f"ht{dc}", tag=f"ht{dc}")
                    nc.gpsimd.dma_start(h16[:], hT_d[dc * 128:(dc + 1) * 128, :])
                    ht.append(h16)
                wkva = [pw.tile([128, KVR + ROPE], F16, name=f"wkva{dc}", tag=f"wkva{dc}") for dc in range(NDC)]
                for dc in range(NDC):
                    nc.sync.dma_start(wkva[dc][:], wkva_d[dc * 128:(dc + 1) * 128, :])
                wqa = [pw.tile([128, QR], F16, name=f"wqa{dc}", tag=f"wqa{dc}") for dc in range(NDC)]
                for dc in range(NDC):
                    nc.sync.dma_start(wqa[dc][:], wqa_d[dc * 128:(dc + 1) * 128, :])

                # ---- A-proj ckv (raw) + sumsq; rope rows raw ----
                pss_kv = psr.tile([1, ST], F32, name="pss", tag="pss")
                for rc in range(NKC):
                    ps = psm.tile([128, ST], F32, name="psA", tag="psA")
                    for dc in range(NDC):
                        nc.tensor.matmul(
                            ps[:], wkva[dc][:, rc * 128:(rc + 1) * 128], ht[dc][:],
                            start=(dc == 0), stop=(dc == NDC - 1))
                    raw = pr.tile([128, ST], F16, name="kraw", tag="kraw")
                    nc.any.tensor_copy(raw[:], ps[:])
                    nc.sync.dma_start(agkv_src[rc * 128:(rc + 1) * 128, :], raw[:])
                    sq = pl.tile([128, ST], F16, name="sq", tag="sq")
                    nc.vector.tensor_mul(sq[:], raw[:], raw[:])
                    nc.tensor.matmul(pss_kv[:], ones_col[:], sq[:],
                                     start=(rc == 0), stop=(rc == NKC - 1))
                psp = psm.tile([ROPE, ST], F32, name="psRope", tag="psRope", bufs=1)
                for dc in range(NDC):
                    nc.tensor.matmul(psp[:], wkva[dc][:, KVR:KVR + ROPE], ht[dc][:],
                                     start=(dc == 0), stop=(dc == NDC - 1))
                kpe_s = pl1.tile([ROPE, ST], F16, name="kpe_s", tag="kpe_s")
                nc.any.tensor_copy(kpe_s[:], psp[:])
                nc.sync.dma_start(agkv_src[KVR:KVR + ROPE, :], kpe_s[:])
                # inv_kv row
                sqv = pl1.tile([1, ST], F32, name="sqv", tag="sqv")
                nc.scalar.activation(sqv[:], pss_kv[:], AF.Sqrt, scale=1.0 / KVR, bias=epst[:])
                inv = pl1.tile([1, ST], F32, name="inv", tag="inv")
                nc.vector.reciprocal_approx_fast(inv[:], sqv[:])
                inv16 = pl1.tile([1, ST], F16, name="inv16", tag="inv16")
                nc.any.tensor_copy(inv16[:], inv[:])
                nc.sync.dma_start(agkv_src[KVR + ROPE:AGKV_R, :], inv16[:])

                # ---- AllGather raw kv activations (early, under q A-proj) ----
                nc.gpsimd.collective_compute(
                    "AllGather", mybir.AluOpType.bypass, replica_groups=GROUPS,
                    ins=[agkv_src.opt()], outs=[agkv_dst.opt()])

                # ---- A-proj q (raw) + sumsq ----
                pss_q = psr.tile([1, ST], F32, name="pss", tag="pss")
                for rc in range(NRC):
                    ps = psm.tile([128, ST], F32, name="psA", tag="psA")
                    for dc in range(NDC):
                        nc.tensor.matmul(
                            ps[:], wqa[dc][:, rc * 128:(rc + 1) * 128], ht[dc][:],
                            start=(dc == 0), stop=(dc == NDC - 1))
                    raw = pr.tile([128, ST], F16, name="qraw", tag="qraw")
                    nc.any.tensor_copy(raw[:], ps[:])
                    nc.sync.dma_start(agq_src[rc * 128:(rc + 1) * 128, :], raw[:])
                    sq = pl.tile([128, ST], F16, name="sq", tag="sq")
                    nc.vector.tensor_mul(sq[:], raw[:], raw[:])
                    nc.tensor.matmul(pss_q[:], ones_col[:], sq[:],
                                     start=(rc == 0), stop=(rc == NRC - 1))
                sqv2 = pl1.tile([1, ST], F32, name="sqv", tag="sqv")
                nc.scalar.activation(sqv2[:], pss_q[:], AF.Sqrt, scale=1.0 / QR, bias=epst[:])
                inv2 = pl1.tile([1, ST], F32, name="inv", tag="inv")
                nc.vector.reciprocal_approx_fast(inv2[:], sqv2[:])
                inv162 = pl1.tile([1, ST], F16, name="inv16", tag="inv16")
                nc.any.tensor_copy(inv162[:], inv2[:])
                nc.sync.dma_start(agq_src[QR:AGQ_R, :], inv162[:])

                # ---- AllGather raw q activations ----
                nc.gpsimd.collective_compute(
                    "AllGather", mybir.AluOpType.bypass, replica_groups=GROUPS,
                    ins=[agq_src.opt()], outs=[agq_dst.opt()])

            # ---------------- stage B on gathered activations ----------------
            with (
                tc.tile_pool(name="wB", bufs=1) as pwb,
                tc.tile_pool(name="gath", bufs=1) as pg,
                tc.tile_pool(name="bc", bufs=1) as pbc,
                tc.tile_pool(name="psB", bufs=1, space="PSUM") as psmb,
                tc.tile_pool(name="psBc", bufs=2, space="PSUM") as psbc,
            ):
                wkvbk = [pwb.tile([128, HPG * NOPE], F16, name=f"wkvbk{rc}", tag=f"wkvbk{rc}") for rc in range(NKC)]
                wkvbv = [pwb.tile([128, HPG * VDIM], F16, name=f"wkvbv{rc}", tag=f"wkvbv{rc}") for rc in range(NKC)]
                for rc in range(NKC):
                    nc.sync.dma_start(wkvbk[rc][:], wkvbk_d[rc * 128:(rc + 1) * 128, :])
                    nc.sync.dma_start(wkvbv[rc][:], wkvbv_d[rc * 128:(rc + 1) * 128, :])
                wqbn = [pwb.tile([128, HPG * NOPE], F16, name=f"wqbn{rc}", tag=f"wqbn{rc}") for rc in range(NRC)]
                wqbr = [pwb.tile([128, HPG * ROPE], F16, name=f"wqbr{rc}", tag=f"wqbr{rc}") for rc in range(NRC)]
                for rc in range(NRC):
                    nc.sync.dma_start(wqbn[rc][:], wqbn_d[rc * 128:(rc + 1) * 128, :])
                    nc.sync.dma_start(wqbr[rc][:], wqbr_d[rc * 128:(rc + 1) * 128, :])

                # gathered kv (raw) + inv rows
                kg = [[pg.tile([128, ST], F16, name=f"kg{s}_{rc}", tag=f"kg{s}_{rc}")
                       for rc in range(NKC)] for s in range(NST)]
                invkv16 = pg.tile([1, S], F16, name="invkv16", tag="invkv16")
                for s in range(NST):
                    for rc in range(NKC):
                        nc.sync.dma_start(kg[s][rc][:], agkv_dst[s, rc * 128:(rc + 1) * 128, :])
                    nc.sync.dma_start(kpe_raw[:, s * ST:(s + 1) * ST],
                                      agkv_dst[s, KVR:KVR + ROPE, :])
                    nc.sync.dma_start(invkv16[:, s * ST:(s + 1) * ST],
                                      agkv_dst[s, KVR + ROPE:AGKV_R, :])

                # broadcast inv_kv to 128 partitions, normalize kg in place
                bckv = pbc.tile([128, S], F16, name="bckv", tag="bckv")
                for s in range(NST):
                    sl = slice(s * ST, (s + 1) * ST)
                    psb = psbc.tile([128, ST], F32, name="psbc", tag="psbc")
                    nc.tensor.matmul(psb[:], ones_row[:], invkv16[:, sl], start=True, stop=True)
                    nc.any.tensor_copy(bckv[:, sl], psb[:])
                for s in range(NST):
                    sl = slice(s * ST, (s + 1) * ST)
                    for rc in range(NKC):
                        nc.vector.tensor_mul(kg[s][rc][:], kg[s][rc][:], bckv[:, sl])

                # ---- kTn: 4 heads x all tokens ----
                for mc in range(HPG):
                    pss4 = [psmb.tile([128, ST], F32, name=f"psB{s}", tag=f"psB{s}", bufs=1)
                            for s in range(NST)]
                    for rc in range(NKC):
                        for s in range(NST):
                            nc.tensor.matmul(
                                pss4[s][:], wkvbk[rc][:, mc * 128:(mc + 1) * 128], kg[s][rc][:],
                                start=(rc == 0), stop=(rc == NKC - 1))
                    for s in range(NST):
                        nc.any.tensor_copy(kTn[mc][:, s * ST:(s + 1) * ST], pss4[s][:])
                # ---- V: natural layout, all token tiles ----
                for s in range(NST):
                    for tt in range(4):
                        ps = psmb.tile([128, HPG * VDIM], F32, name="psB0", tag="psB0", bufs=1)
                        for rc in range(NKC):
                            nc.tensor.matmul(
                                ps[:], kg[s][rc][:, tt * 128:(tt + 1) * 128], wkvbv[rc][:],
                                start=(rc == 0), stop=(rc == NKC - 1))
                        nc.any.tensor_copy(Vn[s * 4 + tt][:], ps[:])

                # ---- k rope (vector; runs early under the q AllGather) ----
                kpe_both = pp.tile([128, S], F16)
                with tc.tile_pool(name="ropek", bufs=1) as prk:
                    cos2 = pbc.tile([2 * ROPE, S], F16, name="cos2", tag="cos2")
                    nc.sync.dma_start(cos2[:], cos2_d[:])
                    sin2 = pbc.tile([2 * ROPE, S], F16, name="sin2", tag="sin2")
                    nc.sync.dma_start(sin2[:], sin2_d[:])
                    HR = ROPE // 2  # 32
                    rot = prk.tile([ROPE, S], F16, name="rotk", tag="rotk")
                    nc.vector.tensor_scalar_mul(rot[0:HR, :], kpe_raw[HR:ROPE, :], -1.0)
                    nc.vector.tensor_copy(rot[HR:ROPE, :], kpe_raw[0:HR, :])
                    nc.vector.tensor_mul(kpe_both[0:ROPE, :], kpe_raw[:], cos2[0:ROPE, :])
                    nc.vector.tensor_mul(rot[:], rot[:], sin2[0:ROPE, :])
                    # kpe duplicated into both partition halves so the rope
                    # matmul's lhsT base_partition can match either q-rope
                    # slice (0 or 64)
                    nc.vector.tensor_add(kpe_both[0:ROPE, :], kpe_both[0:ROPE, :], rot[:])
                    nc.vector.tensor_copy(kpe_both[ROPE:2 * ROPE, :], kpe_both[0:ROPE, :])

                # gathered q (raw) + inv row
                qg = [[pg.tile([128, ST], F16, name=f"qg{s}_{rc}", tag=f"qg{s}_{rc}")
                       for rc in range(NRC)] for s in range(NST)]
                invq16 = pg.tile([1, S], F16, name="invq16", tag="invq16")
                for s in range(NST):
                    for rc in range(NRC):
                        nc.sync.dma_start(qg[s][rc][:], agq_dst[s, rc * 128:(rc + 1) * 128, :])
                    nc.sync.dma_start(invq16[:, s * ST:(s + 1) * ST],
                                      agq_dst[s, QR:AGQ_R, :])
                bcq = pbc.tile([128, S], F16, name="bcq", tag="bcq")
                for s in range(NST):
                    sl = slice(s * ST, (s + 1) * ST)
                    psb = psbc.tile([128, ST], F32, name="psbc", tag="psbc")
                    nc.tensor.matmul(psb[:], ones_row[:], invq16[:, sl], start=True, stop=True)
                    nc.any.tensor_copy(bcq[:, sl], psb[:])

                # ---- q_b: norm fused into the PSUM->SBUF copies ----
                for mc in range(HPG):
                    pss4 = [psmb.tile([128, ST], F32, name=f"psB{s}", tag=f"psB{s}", bufs=1)
                            for s in range(NST)]
                    for rc in range(NRC):
                        for s in range(NST):
                            nc.tensor.matmul(
                                pss4[s][:], wqbn[rc][:, mc * 128:(mc + 1) * 128], qg[s][rc][:],
                                start=(rc == 0), stop=(rc == NRC - 1))
                    for s in range(NST):
                        sl = slice(s * ST, (s + 1) * ST)
                        nc.vector.tensor_mul(qTn[mc][:, sl], pss4[s][:], bcq[:, sl])
                for mc in range(2):
                    pss4 = [psmb.tile([128, ST], F32, name=f"psB{s}", tag=f"psB{s}", bufs=1)
                            for s in range(NST)]
                    for rc in range(NRC):
                        for s in range(NST):
                            nc.tensor.matmul(
                                pss4[s][:], wqbr[rc][:, mc * 128:(mc + 1) * 128], qg[s][rc][:],
                                start=(rc == 0), stop=(rc == NRC - 1))
                    for s in range(NST):
                        sl = slice(s * ST, (s + 1) * ST)
                        nc.vector.tensor_mul(qTr_raw[mc][:, sl], pss4[s][:], bcq[:, sl])

                # ---- q rope (vector, in place on qTr_raw, tail of stage B) ----
                with tc.tile_pool(name="ropeq", bufs=1) as prq:
                    HR = ROPE // 2
                    for i in range(2):
                        rq = prq.tile([128, S], F16, name="rotq", tag="rotq")
                        for hh in range(2):
                            o = hh * ROPE
                            nc.vector.tensor_scalar_mul(
                                rq[o:o + HR, :], qTr_raw[i][o + HR:o + ROPE, :], -1.0)
                            nc.vector.tensor_copy(
                                rq[o + HR:o + ROPE, :], qTr_raw[i][o:o + HR, :])
                        nc.vector.tensor_mul(qTr_raw[i][:], qTr_raw[i][:], cos2[:])
                        nc.vector.tensor_mul(rq[:], rq[:], sin2[:])
                        nc.vector.tensor_add(qTr_raw[i][:], qTr_raw[i][:], rq[:])
                qTr = qTr_raw

            # ---------------- attention (transposed) + per-head AllGather ----------------
            # o-proj weights stream in under the attention compute
            oproj_pool = tc.tile_pool(name="oproj", bufs=1)
            po = oproj_pool.__enter__()
            wo = [po.tile([128, D], F16, name=f"wo{hc}", tag=f"wo{hc}") for hc in range(H)]
            for hc in range(H):
                nc.sync.dma_start(wo[hc][:], wo_d[hc * 128:(hc + 1) * 128, :])

            attnT = [pp.tile([128, S], F16, name=f"attnT{i}", tag=f"attnT{i}") for i in range(HPG)]
            agat_src = [pdr.tile([VDIM, S], F16, name=f"agat_src{h}", tag=f"agat_src{h}")
                        for h in range(HPG)]
            agat_dst = [pdr.tile([NST, VDIM, S], F16, name=f"agat_dst{h}", tag=f"agat_dst{h}")
                        for h in range(HPG)]
            with (
                tc.tile_pool(name="attn", bufs=1) as pat,
                tc.tile_pool(name="ptp", bufs=6) as ptp,
                tc.tile_pool(name="bcr", bufs=2) as pbr,
                tc.tile_pool(name="psS", bufs=3, space="PSUM") as psS,
                tc.tile_pool(name="psR", bufs=2, space="PSUM") as psR,
                tc.tile_pool(name="psA2", bufs=2, space="PSUM") as psA2,
                tc.tile_pool(name="psN", bufs=1, space="PSUM") as psN,
            ):
                if mask_mode == "causal":
                    pmask = [pat.tile([128, ST], F16, name=f"pm{r}", tag=f"pm{r}") for r in range(4)]
                    for r in range(4):
                        nc.sync.dma_start(pmask[r][:], pmask_d[r])
                for h in range(HPG):
                    qtr_t = qTr[h // 2]
                    ro = (h % 2) * ROPE
                    for qb in range(NST):
                        qsl = slice(qb * ST, (qb + 1) * ST)
                        nkt = 4 * (qb + 1) if mask_mode == "causal" else NTT
                        ps_rs = psR.tile([1, ST], F32, name="psrs", tag="psrs")
                        ps_at = psA2.tile([128, ST], F32, name="psat", tag="psat")
                        for kt in range(nkt):
                            ps = psS.tile([128, ST], F32, name="pss", tag="pss")
                            ksl = slice(kt * 128, (kt + 1) * 128)
                            nc.tensor.matmul(ps[:], kTn[h][:, ksl], qTn[h][:, qsl],
                                             start=True, stop=False)
                            nc.tensor.matmul(ps[:], kpe_both[ro:ro + ROPE, ksl],
                                             qtr_t[ro:ro + ROPE, qsl],
                                             start=False, stop=True)
                            if mask_mode == "generic":
                                mt = ptp.tile([128, ST], F32, name="mt", tag="mt")
                                nc.sync.dma_start(mt[:], maskT_d[ksl, qsl])
                                nc.vector.tensor_add(ps[:], ps[:], mt[:])
                            pt = ptp.tile([128, ST], F16, name="pt", tag="pt")
                            nc.scalar.activation(pt[:], ps[:], AF.Exp)
                            if mask_mode == "causal" and kt >= 4 * qb:
                                nc.vector.tensor_mul(pt[:], pt[:], pmask[kt % 4][:])
                            nc.tensor.matmul(ps_rs[:], ones_col[:], pt[:],
                                             start=(kt == 0), stop=(kt == nkt - 1))
                            nc.tensor.matmul(ps_at[:], Vn[kt][:, h * VDIM:(h + 1) * VDIM],
                                             pt[:], start=(kt == 0), stop=(kt == nkt - 1))
                        # normalize: broadcast raw row-sums, fast-reciprocal on
                        # the [128,512] broadcast, one fused multiply
                        rs16 = pat.tile([1, ST], F16, name="rs16", tag="rs16")
                        nc.any.tensor_copy(rs16[:], ps_rs[:])
                        psb = psN.tile([128, ST], F32, name="psn", tag="psn")
                        nc.tensor.matmul(psb[:], ones_row[:], rs16[:], start=True, stop=True)
                        bcr = pbr.tile([128, ST], F32, name="bcr", tag="bcr")
                        nc.vector.reciprocal_approx_fast(bcr[:], psb[:])
                        nc.vector.tensor_mul(attnT[h][:, qsl], ps_at[:], bcr[:])
                    # ship this head's attnT while later heads compute
                    nc.sync.dma_start(agat_src[h][:], attnT[h][:])
                    nc.gpsimd.collective_compute(
                        "AllGather", mybir.AluOpType.bypass, replica_groups=GROUPS,
                        ins=[agat_src[h].opt()], outs=[agat_dst[h].opt()])

            # ------- o-proj: slice own tokens from gathered heads, full contract -------
            with (
                tc.tile_pool(name="oloop", bufs=3) as pol,
                tc.tile_pool(name="psO", bufs=2, space="PSUM") as psO,
            ):
                pid = nc.partition_id()
                toff = nc.snap((pid % NST) * ST, donate=True)
                atg = [po.tile([128, ST], F16, name=f"atg{hc}", tag=f"atg{hc}") for hc in range(H)]
                for hc in range(H):
                    nc.gpsimd.dma_start(
                        atg[hc][:],
                        agat_dst[hc % 4][hc // 4, :, bass.ds(toff, ST)])
                for ncol in range(4):
                    csl = slice(ncol * ST, (ncol + 1) * ST)
                    for tl in range(4):
                        ps = psO.tile([128, ST], F32, name="pso", tag="pso")
                        for hc in range(H):
                            nc.tensor.matmul(ps[:], atg[hc][:, tl * 128:(tl + 1) * 128],
                                             wo[hc][:, csl],
                                             start=(hc == 0), stop=(hc == H - 1))
                        ot = pol.tile([128, ST], F32, name="ot", tag="ot")
                        nc.any.tensor_copy(ot[:], ps[:])
                        nc.sync.dma_start(o_d[tl * 128:(tl + 1) * 128, csl], ot[:])
            oproj_pool.__exit__(None, None, None)

    # populate .instr bytes for extended-inst InstISA subclasses (the
    # custom-DVE reciprocal_approx_fast) — raw Bass skips this pass
    mybir.codegen_inst_isa_subclasses(nc)
    _split_multi_waits(nc)
    return nc


_CACHE = {}


def _get_program(mask_mode):
    if mask_mode not in _CACHE:
        _CACHE[mask_mode] = _build_program(mask_mode)
    return _CACHE[mask_mode]


def _host_prep(hidden_states, attention_mask, position_ids, w_qa, qa_ln_w, w_qb,
               w_kva, kva_ln_w, w_kvb, w_o):
    f16 = np.float16
    mask2d = np.asarray(attention_mask, np.float32).reshape(S, S)
    causal_ref = np.triu(np.full((S, S), -1e9, np.float32), k=1)
    if np.array_equal(mask2d, causal_ref):
        mask_mode = "causal"
    elif not mask2d.any():
        mask_mode = "none"
    else:
        mask_mode = "generic"

    # weight prep: fold RMSNorm gains into B-projections, SCALE into q side
    w_qb_eff = (np.asarray(w_qb, np.float32) * np.asarray(qa_ln_w, np.float32)[:, None]) * SCALE
    w_kvb_eff = np.asarray(w_kvb, np.float32) * np.asarray(kva_ln_w, np.float32)[:, None]
    wqb3 = w_qb_eff.reshape(QR, H, QHD)
    wkvb3 = w_kvb_eff.reshape(KVR, H, NOPE + VDIM)

    pos = np.asarray(position_ids).astype(np.int64)
    inv_freq = 1.0 / (THETA ** (np.arange(0, ROPE, 2, dtype=np.float32) / ROPE))
    t = np.arange(S, dtype=np.float32)
    freqs = np.outer(t, inv_freq)
    emb = np.concatenate([freqs, freqs], axis=-1)   # [S, ROPE]
    cosT = np.cos(emb)[pos].T.astype(f16)           # [ROPE, S]
    sinT = np.sin(emb)[pos].T.astype(f16)
    cos2 = np.ascontiguousarray(np.concatenate([cosT, cosT], axis=0))  # [128, S]
    sin2 = np.ascontiguousarray(np.concatenate([sinT, sinT], axis=0))

    # causal keep-mask patterns for the transposed diagonal tiles:
    # keep iff 128*r + ki <= qj  (r = kt % 4)
    ki = np.arange(128)[:, None]
    qj = np.arange(ST)[None, :]
    pmaskT = np.stack([(128 * r + ki <= qj) for r in range(4)]).astype(f16)

    wqa16 = np.asarray(w_qa, np.float32).astype(f16)
    wkva16 = np.asarray(w_kva, np.float32).astype(f16)

    hiddenT = [np.ascontiguousarray(np.asarray(hidden_states[b], np.float32).T)
               for b in range(B)]
    wo_full = np.asarray(w_o, np.float32).astype(f16)

    in_maps = []
    for c in range(8):
        b, g = divmod(c, 4)
        hs = range(g * HPG, (g + 1) * HPG)
        m = {
            "hiddenT": np.ascontiguousarray(hiddenT[b][:, g * ST:(g + 1) * ST]),
            "wqa": wqa16,
            "wkva": wkva16,
            "wqbn": np.ascontiguousarray(
                np.concatenate([wqb3[:, h, :NOPE] for h in hs], axis=1)).astype(f16),
            "wqbr": np.ascontiguousarray(
                np.concatenate([wqb3[:, h, NOPE:] for h in hs], axis=1)).astype(f16),
            "wkvbk": np.ascontiguousarray(
                np.concatenate([wkvb3[:, h, :NOPE] for h in hs], axis=1)).astype(f16),
            "wkvbv": np.ascontiguousarray(
                np.concatenate([wkvb3[:, h, NOPE:] for h in hs], axis=1)).astype(f16),
            "wo": wo_full,
            "cos2": cos2,
            "sin2": sin2,
        }
        if mask_mode == "causal":
            m["pmaskT"] = pmaskT
        if mask_mode == "generic":
            m["maskT"] = np.ascontiguousarray(mask2d.T)
        in_maps.append(m)
    return mask_mode, in_maps


def kernel(hidden_states, attention_mask, position_ids, w_qa, qa_ln_w, w_qb,
           w_kva, kva_ln_w, w_kvb, w_o, _want_trace=False, _trace_kwargs=None):
    mask_mode, in_maps = _host_prep(
        hidden_states, attention_mask, position_ids, w_qa, qa_ln_w, w_qb,
        w_kva, kva_ln_w, w_kvb, w_o)
    nc = _get_program(mask_mode)
    kwargs = {}
    if _want_trace:
        kwargs.update(trace=True, **(_trace_kwargs or {}))
    res = run_bass_kernel_spmd(nc, in_maps, list(range(8)), **kwargs)
    out = np.empty((B, S, D), np.float32)
    for c in range(8):
        b, g = divmod(c, 4)
        out[b, g * ST:(g + 1) * ST, :] = res.results[c]["o_part"]
    if _want_trace:
        kernel._last_result = res
    return out
